# revision 1
# baseline (speedup 1.0000x reference)
"""Trainium2 Bass kernel for a local-attention transformer block (v2).

Per batch element (one NeuronCore each, 8 cores):
  y   = LN(x) @ diag(arow) + crow  -- folded host-side into per-core qkv weights
  q,k,v = y @ qkv_w' + qkv_b'      (heads=8, d=64), fp8 DoubleRow GEMM
  attn: each 128-token window attends to [prev|cur|next] windows
  x1  = x + attn @ proj_w + proj_b (fp8 DoubleRow)
  out = x1 + gelu(LN2(x1) @ w1 + b1') @ w2 + b2 (fp8 DoubleRow, ln2 folded)

Layout:
  - x, x1, out: token-major [128 tok, 512] fp32 tiles; LN via bn_stats and
    rsqrt = exp(-0.5 ln(var+eps)) so LN shares the natural_log_exp ACT table
    with the attention exp.
  - y, h2 fp16 -> DMA-xbar transposed feature-major, then cast to fp8
    DoubleRow pair tiles [128, 2, 512].
  - q,k feature-major fp16 [128 feat(head pair), 512 tok] chunks.
  - v token-major fp16 [128 tok, 8 heads, 65] with ones column folded in so
    the attention-value matmul also yields softmax denominators (row 64).
  - sim computed key-block-major: for key block j one wide matmul
    [64 feat, 128 keys]^T @ [64 feat, <=384 qtok] -> PSUM [128, 384], the two
    heads of a chunk issued back-to-back on different PE row groups
    (concurrent). exp on ACT -> E[j][h] fp16 [128, 384].
  - AV: per (head, group) 6 wide matmuls accumulate v_aug^T @ E[j] slices at
    column offsets into one PSUM bank [65, 512]; ordering chosen so each
    matmul's region has uniform has_written state.
  - softmax normalize: DVE reciprocal of the sums row + gpsimd
    partition_broadcast + DVE multiply straight into fp8 proj-input tiles.
  - MLP lagged two groups behind attention so gelu table loads batch up.
"""

import numpy as np
from contextlib import ExitStack

import concourse.bass as bass
import concourse.tile as tile
from concourse import bacc, mybir
from concourse import bass_utils

F32 = mybir.dt.float32
F16 = mybir.dt.float16
F8 = mybir.dt.float8e4
AF = mybir.ActivationFunctionType
AL = mybir.AluOpType
DR = mybir.MatmulPerfMode.DoubleRow

DIM = 512
HEADS = 8
HD = 64
FF = 2048
WIN = 128
B = 8
NTOK = 8192
EPS = 1e-5
GRP = 512  # tokens per group (4 windows)
TS = 128   # token slice (partition tile)


def _bcast_row(dram_ap, offset, n):
    """AP reading dram vector [n] broadcast across 128 partitions."""
    return bass.AP(tensor=dram_ap.tensor, offset=offset, ap=[[0, 128], [1, n]])


def _col_view(dram_ap, offset, ncol):
    """AP reading dram vector [128*ncol] as [128, ncol] feature-major columns."""
    return bass.AP(tensor=dram_ap.tensor, offset=offset, ap=[[1, 128], [128, ncol]])


def build(n_tok=NTOK):
    n_groups = n_tok // GRP
    n_blocks = n_tok // WIN
    nc = bacc.Bacc("TRN2", target_bir_lowering=False, debug=False)

    x_d = nc.dram_tensor("x", [n_tok, DIM], F32, kind="ExternalInput")
    qkvw_d = nc.dram_tensor("qkvw", [2, 128, 2, 3 * DIM], F8, kind="ExternalInput")
    qkvb_d = nc.dram_tensor("qkvb", [3 * DIM], F32, kind="ExternalInput")
    projw_d = nc.dram_tensor("projw", [2, 128, 2, DIM], F8, kind="ExternalInput")
    projb_d = nc.dram_tensor("projb", [DIM], F32, kind="ExternalInput")
    w1_d = nc.dram_tensor("w1", [2, 128, 2, FF], F8, kind="ExternalInput")
    b1_d = nc.dram_tensor("b1", [FF], F32, kind="ExternalInput")
    w2_d = nc.dram_tensor("w2", [8, 128, 2, DIM], F8, kind="ExternalInput")
    b2_d = nc.dram_tensor("b2", [DIM], F32, kind="ExternalInput")
    out_d = nc.dram_tensor("out", [n_tok, DIM], F32, kind="ExternalOutput")

    with tile.TileContext(nc) as tc:
        with ExitStack() as ctx:
            consts = ctx.enter_context(tc.tile_pool(name="consts", bufs=1))
            xp = ctx.enter_context(tc.tile_pool(name="xp", bufs=4))
            yp = ctx.enter_context(tc.tile_pool(name="yp", bufs=1))
            ytp = ctx.enter_context(tc.tile_pool(name="ytp", bufs=3))
            yt8p = ctx.enter_context(tc.tile_pool(name="yt8p", bufs=3))
            qp = ctx.enter_context(tc.tile_pool(name="qp", bufs=2))
            kp = ctx.enter_context(tc.tile_pool(name="kp", bufs=2))
            vp = ctx.enter_context(tc.tile_pool(name="vp", bufs=3))
            ep = ctx.enter_context(tc.tile_pool(name="ep", bufs=1))
            rp = ctx.enter_context(tc.tile_pool(name="rp", bufs=1))
            rbp = ctx.enter_context(tc.tile_pool(name="rbp", bufs=1))
            a8p = ctx.enter_context(tc.tile_pool(name="a8p", bufs=2))
            x1p = ctx.enter_context(tc.tile_pool(name="x1p", bufs=3))
            h2p = ctx.enter_context(tc.tile_pool(name="h2p", bufs=1))
            h2tp = ctx.enter_context(tc.tile_pool(name="h2tp", bufs=3))
            gp = ctx.enter_context(tc.tile_pool(name="gp", bufs=1))
            tp = ctx.enter_context(tc.tile_pool(name="tp", bufs=2))
            ps_g = ctx.enter_context(tc.tile_pool(name="ps_g", bufs=4, space="PSUM"))
            ps_s = ctx.enter_context(tc.tile_pool(name="ps_s", bufs=1, space="PSUM"))
            ps_a = ctx.enter_context(tc.tile_pool(name="ps_a", bufs=2, space="PSUM"))

            # ---- constants ----
            qkvw_sb = []
            for kt in range(2):
                t = consts.tile([128, 2, 3 * DIM], F8, name=f"qkvw{kt}")
                nc.sync.dma_start(t[:], qkvw_d[kt])
                qkvw_sb.append(t)
            projw_sb = []
            for kt in range(2):
                t = consts.tile([128, 2, DIM], F8, name=f"projw{kt}")
                nc.sync.dma_start(t[:], projw_d[kt])
                projw_sb.append(t)
            w1_sb = []
            for kt in range(2):
                t = consts.tile([128, 2, FF], F8, name=f"w1_{kt}")
                nc.sync.dma_start(t[:], w1_d[kt])
                w1_sb.append(t)
            w2_sb = []
            for p in range(8):
                t = consts.tile([128, 2, DIM], F8, name=f"w2_{p}")
                nc.sync.dma_start(t[:], w2_d[p])
                w2_sb.append(t)

            projb_bc = consts.tile([128, DIM], F32, name="projb_bc")
            nc.sync.dma_start(projb_bc[:], _bcast_row(projb_d.ap(), 0, DIM))
            b2_bc = consts.tile([128, DIM], F32, name="b2_bc")
            nc.sync.dma_start(b2_bc[:], _bcast_row(b2_d.ap(), 0, DIM))
            vb_bc = consts.tile([128, DIM], F32, name="vb_bc")
            nc.sync.dma_start(vb_bc[:], _bcast_row(qkvb_d.ap(), 2 * DIM, DIM))
            qkb_sb = consts.tile([128, 8], F32, name="qkb_sb")
            nc.sync.dma_start(qkb_sb[:], _col_view(qkvb_d.ap(), 0, 8))
            b1_sb = consts.tile([128, 16], F32, name="b1_sb")
            nc.sync.dma_start(b1_sb[:], _col_view(b1_d.ap(), 0, 16))
            eps_t = consts.tile([128, 1], F32, name="eps_t")
            nc.vector.memset(eps_t[:], EPS)

            stages = {}
            E = {}
            e_padded = set()

            def ln_tiles(pref, g, src_tiles, out_pool, out_tag):
                """LN over last dim of 4 token-major tiles -> fp16 tiles.

                rsqrt(var+eps) on DVE only: quake-style int seed
                (0x5f3759df - (bits >> 1), exact via ~x - ~C in mod-2^32)
                plus two Newton steps y <- y*(1.5 - 0.5*v*y^2). No ACT
                tables touched (keeps exp/gelu sets resident)."""
                mv4 = tp.tile([128, 4, 2], F32, name=f"{pref}mv_{g}", tag=f"{pref}mv")
                for t in range(4):
                    stats = tp.tile([128, 6], F32, name=f"{pref}st_{g}_{t}", tag=f"{pref}st{t}")
                    nc.vector.bn_stats(stats[:], src_tiles[t][:])
                    nc.vector.bn_aggr(mv4[:, t, :], stats[:])
                var4 = tp.tile([128, 4], F32, name=f"{pref}v4_{g}", tag=f"{pref}v4")
                nc.vector.tensor_scalar(var4[:], mv4[:, :, 1], EPS, None, op0=AL.add)
                y0 = tp.tile([128, 4], F32, name=f"{pref}y0_{g}", tag=f"{pref}y0")
                y0i = y0[:].bitcast(mybir.dt.uint32)
                nc.vector.tensor_scalar(y0i, var4[:].bitcast(mybir.dt.uint32),
                                        1, 0xFFFFFFFF,
                                        op0=AL.logical_shift_right, op1=AL.bitwise_xor)
                nc.vector.tensor_scalar(y0i, y0i, 0xA0C8A620, None, op0=AL.subtract)
                rs4 = tp.tile([128, 4], F32, name=f"{pref}rs_{g}", tag=f"{pref}rs")
                t1 = tp.tile([128, 4], F32, name=f"{pref}t1_{g}", tag=f"{pref}t1")
                for it in range(2):
                    src = y0 if it == 0 else rs4
                    nc.vector.tensor_tensor(t1[:], src[:], src[:], op=AL.mult)
                    nc.vector.tensor_tensor(t1[:], t1[:], var4[:], op=AL.mult)
                    nc.vector.tensor_scalar(t1[:], t1[:], -0.5, 1.5,
                                            op0=AL.mult, op1=AL.add)
                    nc.vector.tensor_tensor(rs4[:], t1[:], src[:], op=AL.mult)
                outs = []
                for t in range(4):
                    yt_ = out_pool.tile([128, DIM], F8, name=f"{pref}y_{g}_{t}", tag=f"{out_tag}{t}")
                    nc.vector.tensor_scalar(yt_[:], src_tiles[t][:], mv4[:, t, 0:1],
                                            rs4[:, t:t + 1],
                                            op0=AL.subtract, op1=AL.mult)
                    outs.append(yt_)
                return outs

            def transpose_words(pref, g, y_t, tpool, ttag):
                """4x [128tok,512feat] fp8 -> 2 word tiles [128 wpart, 512 tok]
                (each fp16 word = feature pair (2w, 2w+1)) via DMA-xbar on the
                fp16 bitcast view. Contraction mapping (p,i) -> 2p+i."""
                tw = []
                for kt in range(2):
                    t16 = tpool.tile([128, GRP], F16, name=f"{pref}Tw_{g}_{kt}", tag=f"{ttag}{kt}")
                    for t in range(4):
                        src = y_t[t][:].bitcast(F16)
                        nc.sync.dma_start_transpose(
                            t16[:, t * 128:(t + 1) * 128],
                            src[:, kt * 128:(kt + 1) * 128])
                    tw.append(t16)
                return tw

            def word_rhs(tw):
                """Word tile -> DoubleRow rhs AP [128, 2, 512] (byte-stride)."""
                return tw[:].bitcast(F8).rearrange("p (t two) -> p two t", two=2)

            def stage_ln_y(g):
                """x load + LN1 -> y -> yT8 fp8 pair tiles (no tensor engine)."""
                x_t = []
                for t in range(4):
                    xt = xp.tile([128, DIM], F32, name=f"x_{g}_{t}", tag=f"x{t}")
                    nc.sync.dma_start(xt[:], x_d[(g * 4 + t) * 128:(g * 4 + t + 1) * 128, :])
                    x_t.append(xt)
                y_t = ln_tiles("a", g, x_t, yp, "y")
                yT8w = transpose_words("y", g, y_t, ytp, "yTw")
                yT8g = []
                for kt in range(2):
                    tg = yt8p.tile([128, 2, GRP], F8, name=f"yT8g_{g}_{kt}", tag=f"yT8g{kt}")
                    nc.vector.tensor_scalar(tg[:], word_rhs(yT8w[kt]), 0.0, None,
                                            op0=AL.add)
                    yT8g.append(tg)
                stages[g] = {"x": x_t, "yT8w": yT8w, "yT8g": yT8g}

            def qkv_mm(g):
                st = stages[g]
                yT8w = st["yT8w"]
                yT8g = st["yT8g"]
                # Q,K: out chunks m=0..7 feature-major [128 feat(head pair), 512 tok]
                q_t, k_t = [], []
                for m in range(8):
                    P = ps_g.tile([128, GRP], F32, name=f"Pqk_{g}_{m}", tag="gemm")
                    for kt in range(2):
                        nc.tensor.matmul(P[:], qkvw_sb[kt][:, :, m * 128:(m + 1) * 128],
                                         word_rhs(yT8w[kt]), start=(kt == 0), stop=(kt == 1),
                                         perf_mode=DR)
                    pool = qp if m < 4 else kp
                    nm = f"q_{g}_{m}" if m < 4 else f"k_{g}_{m-4}"
                    tg = f"q{m}" if m < 4 else f"k{m-4}"
                    sb = pool.tile([128, GRP], F16, name=nm, tag=tg)
                    nc.scalar.activation(sb[:], P[:], AF.Identity, bias=qkb_sb[:, m:m + 1])
                    (q_t if m < 4 else k_t).append(sb)
                st["q"], st["k"] = q_t, k_t
                # V token-major with ones column: [128 tok, 8, 65]
                v_t = []
                for t in range(4):
                    P = ps_g.tile([128, DIM], F32, name=f"Pv_{g}_{t}", tag="gemm")
                    for kt in range(2):
                        nc.tensor.matmul(P[:], yT8g[kt][:, :, t * 128:(t + 1) * 128],
                                         qkvw_sb[kt][:, :, 2 * DIM:3 * DIM],
                                         start=(kt == 0), stop=(kt == 1), perf_mode=DR)
                    vt = vp.tile([128, HEADS, HD + 1], F16, name=f"v_{g}_{t}", tag=f"v{t}")
                    nc.vector.memset(vt[:, :, HD:HD + 1], 1.0)
                    nc.vector.tensor_tensor(
                        vt[:, :, 0:HD],
                        P[:].rearrange("p (h d) -> p h d", h=HEADS),
                        vb_bc[:].rearrange("p (h d) -> p h d", h=HEADS),
                        op=AL.add)
                    v_t.append(vt)
                st["v"] = v_t

            def sim_e(j):
                """sim + exp for key block j: E[j][h] [128 keys, 384] fp16.

                Column d*128.. of E[j] covers q window j-1+d (d=0,1,2)."""
                gj, s = divmod(j, 4)
                t_lo = max(0, (j - 1) * WIN)
                t_hi = min(n_tok, (j + 2) * WIN)
                # split q span at group boundaries
                spans = []
                t0 = t_lo
                while t0 < t_hi:
                    gq = t0 // GRP
                    t1 = min(t_hi, (gq + 1) * GRP)
                    spans.append((gq, t0, t1))
                    t0 = t1
                Ej = {}
                for c in range(4):
                    ps_pair = []
                    for half in (0, 64):
                        h = 2 * c + (half // 64)
                        Ps = ps_s.tile([128, 3 * WIN], F32, name=f"Ps_{j}_{h}", tag=f"sim{half//64}")
                        ps_pair.append(Ps)
                    # issue the two halves' matmuls interleaved per span so the
                    # PE overlaps them on different row groups
                    for (gq, s0, s1) in spans:
                        for hi, half in enumerate((0, 64)):
                            st_j = stages[gj]
                            st_q = stages[gq]
                            nc.tensor.matmul(
                                ps_pair[hi][:, s0 - (j - 1) * WIN:s1 - (j - 1) * WIN],
                                st_j["k"][c][half:half + 64, s * 128:(s + 1) * 128],
                                st_q["q"][c][half:half + 64, s0 - gq * GRP:s1 - gq * GRP],
                                start=True, stop=True)
                    for hi, half in enumerate((0, 64)):
                        h = 2 * c + hi
                        wid = 4 * WIN if j % 2 == 1 else 3 * WIN
                        tag = f"E{h}_{j % 6}"
                        Et = ep.tile([128, wid], F16, name=f"E_{j}_{h}", tag=tag)
                        if j % 2 == 1:
                            # zero pad [384:512]: the widest AV matmul reads it
                            # to claim the whole PSUM bank with one overwrite.
                            # On gpsimd: the DVE queue is backlogged with LN
                            # work and would stall the first AV matmul.
                            nc.gpsimd.memset(Et[:, 3 * WIN:4 * WIN], 0.0)
                        lo = t_lo - (j - 1) * WIN
                        hi_ = t_hi - (j - 1) * WIN
                        nc.scalar.activation(Et[:, lo:hi_], ps_pair[hi][:, lo:hi_],
                                             AF.Exp, scale=float(HD) ** -0.5)
                        Ej[h] = Et
                E[j] = Ej

            def attn(gm):
                """AV + normalize + proj + LN2 + h2 transpose for group gm."""
                # sim/E for the key blocks this group needs
                j_lo = 4 * gm if gm == 0 else 4 * gm + 1
                for j in range(j_lo, min(4 * gm + 5, n_blocks)):
                    sim_e(j)
                # j = 4gm+1 goes first: its E tile is zero-padded to 512
                # so its start=True matmul overwrites the whole PSUM bank
                # (HW clears has_written per written region), then the other
                # key blocks accumulate.
                j1 = 4 * gm + 1
                js = [j1] + [j for j in range(4 * gm - 1, 4 * gm + 5)
                             if 0 <= j < n_blocks and j != j1]
                attn8 = []
                for kt in range(2):
                    t = a8p.tile([128, 2, GRP], F8, name=f"attn8_{gm}_{kt}", tag=f"attn8{kt}")
                    attn8.append(t)
                for h in range(HEADS):
                    av = ps_a.tile([HD + 1, GRP], F32, name=f"Pav_{gm}_{h}", tag="av")
                    for ji, j in enumerate(js):
                        gj, s = divmod(j, 4)
                        if j == j1:
                            lc0, lc1, ec0, ec1 = 0, GRP, 0, GRP
                        else:
                            w_lo = max(j - 1, 4 * gm)
                            w_hi = min(j + 1, 4 * gm + 3)
                            lc0 = (w_lo - 4 * gm) * WIN
                            lc1 = (w_hi + 1 - 4 * gm) * WIN
                            ec0 = (w_lo - (j - 1)) * WIN
                            ec1 = (w_hi + 1 - (j - 1)) * WIN
                        nc.tensor.matmul(
                            av[:, lc0:lc1],
                            stages[gj]["v"][s][:, h, :],
                            E[j][h][:, ec0:ec1],
                            start=(ji == 0), stop=(ji == len(js) - 1))
                    # normalize: sums row -> SBUF partition 0 (ACT copy, shift
                    # HW-proven; approx recip misreads PSUM so keep it SBUF),
                    # fast reciprocal, then gpsimd broadcast across 64 parts
                    s_t = rp.tile([1, GRP], F32, name=f"s_{gm}_{h}", tag=f"s{h % 2}")
                    nc.scalar.activation(s_t[0:1, :], av[64:65, :], AF.Copy)
                    r_t = rp.tile([1, GRP], F32, name=f"r_{gm}_{h}", tag=f"r{h % 2}")
                    nc.vector.reciprocal_approx_fast(r_t[0:1, :], s_t[0:1, :])
                    rbc = rbp.tile([HD, GRP], F32, name=f"rb_{gm}_{h}", tag=f"rb{h % 2}")
                    nc.gpsimd.partition_broadcast(rbc[:], r_t[0:1, :], channels=HD)
                    kt, i, half = h // 4, (h // 2) % 2, 64 * (h % 2)
                    nc.vector.tensor_tensor(
                        attn8[kt][half:half + 64, i, :], av[0:64, :], rbc[:], op=AL.mult)
                # proj + residual -> x1 token-major
                st = stages[gm]
                x1_t = []
                for t in range(4):
                    P = ps_g.tile([128, DIM], F32, name=f"Ppr_{gm}_{t}", tag="gemm")
                    for kt in range(2):
                        nc.tensor.matmul(P[:], attn8[kt][:, :, t * 128:(t + 1) * 128],
                                         projw_sb[kt][:], start=(kt == 0), stop=(kt == 1),
                                         perf_mode=DR)
                    x1 = x1p.tile([128, DIM], F32, name=f"x1_{gm}_{t}", tag=f"x1{t}")
                    nc.vector.tensor_tensor(x1[:], P[:], projb_bc[:], op=AL.add)
                    nc.vector.tensor_tensor(x1[:], x1[:], st["x"][t][:], op=AL.add)
                    x1_t.append(x1)
                st["x1"] = x1_t
                h2_t = ln_tiles("b", gm, x1_t, h2p, "h2")
                st["h2T8w"] = transpose_words("h", gm, h2_t, h2tp, "hTw")

            def mlp(gm):
                st = stages[gm]
                h2T8w = st["h2T8w"]
                gel = []
                for m in range(16):
                    P = ps_g.tile([128, GRP], F32, name=f"Pm1_{gm}_{m}", tag="gemm")
                    for kt in range(2):
                        nc.tensor.matmul(P[:], w1_sb[kt][:, :, m * 128:(m + 1) * 128],
                                         word_rhs(h2T8w[kt]), start=(kt == 0), stop=(kt == 1),
                                         perf_mode=DR)
                    if m % 2 == 0:
                        gl = gp.tile([128, 2, GRP], F8, name=f"gel_{gm}_{m//2}", tag=f"gel{m//2}")
                        gel.append(gl)
                    nc.scalar.activation(gel[m // 2][:, m % 2, :], P[:],
                                         AF.Gelu, bias=b1_sb[:, m:m + 1])
                for t in range(4):
                    P = ps_g.tile([128, DIM], F32, name=f"Pm2_{gm}_{t}", tag="gemm")
                    for p in range(8):
                        nc.tensor.matmul(P[:], gel[p][:, :, t * 128:(t + 1) * 128],
                                         w2_sb[p][:], start=(p == 0), stop=(p == 7),
                                         perf_mode=DR)
                    x1t = st["x1"][t]
                    nc.vector.tensor_tensor(x1t[:], x1t[:], b2_bc[:], op=AL.add)
                    nc.vector.tensor_tensor(x1t[:], x1t[:], P[:], op=AL.add)
                    nc.sync.dma_start(out_d[(gm * 4 + t) * 128:(gm * 4 + t + 1) * 128, :], x1t[:])
                # free references no longer needed
                del st["h2T8w"]

            # software pipeline:
            #   iter g: qkv_mm(g) [uses yT8(g)], prefetch ln_y(g+1),
            #           attn(g-1), mlp in pairs two groups back.
            stage_ln_y(0)
            if n_groups > 1:
                stage_ln_y(1)
            mlp_next = 0
            for g in range(n_groups):
                qkv_mm(g)
                if g + 2 < n_groups:
                    stage_ln_y(g + 2)
                if g >= 1:
                    attn(g - 1)
                if g >= 3 and g % 2 == 1:
                    while mlp_next <= g - 2:
                        mlp(mlp_next)
                        mlp_next += 1
            attn(n_groups - 1)
            while mlp_next < n_groups:
                mlp(mlp_next)
                mlp_next += 1

    nc.compile()
    return nc


_cache = {}


def _get_nc(n_tok):
    if n_tok not in _cache:
        _cache[n_tok] = build(n_tok)
    return _cache[n_tok]


def _dr_pack(W, f8):
    """[K, M] -> [K//256, 128, 2, M] fp8 DoubleRow layout (k = kt*256+i*128+p)."""
    K, M = W.shape
    return np.ascontiguousarray(
        W.reshape(K // 256, 2, 128, M).transpose(0, 2, 1, 3)).astype(f8)


def _dr_pack_w(W, f8):
    """[K, M] -> [K//256, 128, 2, M] word-order DoubleRow (k = kt*256+2p+i)."""
    K, M = W.shape
    return np.ascontiguousarray(W.reshape(K // 256, 128, 2, M)).astype(f8)


def _prep_in_maps(inputs):
    return _prep(**inputs)


def _prep(x, t_emb, ln1_g, ln1_b, qkv_w, qkv_b, proj_w, proj_b,
          ln2_g, ln2_b, mlp_w1, mlp_b1, mlp_w2, mlp_b2, time_w, time_b):
    import ml_dtypes
    f8 = ml_dtypes.float8_e4m3

    x = np.asarray(x, dtype=np.float32)
    t_emb = np.asarray(t_emb, np.float32)
    qkv_w = np.asarray(qkv_w, np.float32)
    qkv_b = np.asarray(qkv_b, np.float32)

    # host: modulation rows, folded into per-batch qkv weights/bias
    s = t_emb / (1.0 + np.exp(-t_emb))           # silu
    ss = s @ np.asarray(time_w, np.float32) + np.asarray(time_b, np.float32)
    scale, shift = ss[:, :DIM], ss[:, DIM:]
    g1 = np.asarray(ln1_g, np.float32)
    be1 = np.asarray(ln1_b, np.float32)
    arow = g1[None, :] * (1.0 + scale)                      # [B, 512]
    crow = be1[None, :] * (1.0 + scale) + shift             # [B, 512]
    # fold ln2 gamma/beta into mlp_w1/b1
    g2 = np.asarray(ln2_g, np.float32)
    be2 = np.asarray(ln2_b, np.float32)
    w1f = np.asarray(mlp_w1, np.float32) * g2[:, None]
    b1f = be2 @ np.asarray(mlp_w1, np.float32) + np.asarray(mlp_b1, np.float32)

    projw8 = _dr_pack(np.asarray(proj_w, np.float32), f8)
    w18 = _dr_pack_w(w1f, f8)
    w28 = _dr_pack(np.asarray(mlp_w2, np.float32), f8)
    projb = np.asarray(proj_b, np.float32)
    b2 = np.asarray(mlp_b2, np.float32)

    in_maps = []
    nb = x.shape[0]
    for b in range(nb):
        qkvw_b = arow[b][:, None] * qkv_w               # [512, 1536]
        qkvb_b = qkv_b + crow[b] @ qkv_w                # [1536]
        in_maps.append({
            "x": np.ascontiguousarray(x[b]),
            "qkvw": _dr_pack_w(qkvw_b, f8), "qkvb": qkvb_b,
            "projw": projw8, "projb": projb,
            "w1": w18, "b1": b1f, "w2": w28, "b2": b2,
        })
    return in_maps


def kernel(**inputs):
    in_maps = _prep_in_maps(inputs)
    n_tok = in_maps[0]["x"].shape[0]
    nc = _get_nc(n_tok)
    nb = len(in_maps)
    res = bass_utils.run_bass_kernel_spmd(nc, in_maps, core_ids=list(range(nb)))
    out = np.stack([res.results[b]["out"] for b in range(nb)], axis=0)
    return out

